# revision 20
# baseline (speedup 1.0000x reference)
"""Trainium2 Bass kernel for nn_Block_38517266710836.

reference pipeline: channel mixer -> STFT (hann 2048, hop 1024) -> per-frame
recurrence out[f] = (spec[f] + out[f-1]) * transfer -> iSTFT (hann synthesis)
-> overlap-add -> gain -> tanh.

Sharding: 8 cores, data-parallel over (batch, channel-half): core c handles
batch c//2, mixed channels [32*(c%2), +32). Each core receives its batch's
full 64-channel input (the mixer contracts channels) and writes 32 rows.

Pipelined single-pass program per core: mixer chunks, forward-DFT frame
batches, the DVE recurrence scan, and the inverse/overlap-add phase are
interleaved so the scan and evictions hide under PE matmul work.  Forward
evictions and corner-turn copies run on ScalarE (DVE is reserved for the
scan), weights stream on the gpsimd DMA queue, x/y on the sync queue, and
PSUM pools are shared across phases to fit the 8-bank budget.
"""

import numpy as np

WINDOW = 2048
STEP = 1024
CPD = 64
BATCH = 4
TIME = 65536
FRAMES = 64
NJ = 16              # per-frame time chunks (fwd contraction blocks)
NM = 16              # spectral slot chunks
DSH = 32             # mixed channels per core
GCH = TIME // 128    # 512 global 128-sample chunks
GPAD = GCH + 16      # + zero pad (frame 63 reaches t=66560; extra width so
                     # the forward rhs slice [base, base+2048) stays in-bounds)
FC = 4               # frame chunks for the scan layout
FW = 16              # frames per chunk
CB = 17              # chain block: 1 inject/reset col + 16 frame cols
SPECW = NM * DSH * CB  # 8704 free cols per fc block


def _hann(n):
    return (0.5 - 0.5 * np.cos(2.0 * np.pi * np.arange(n) / n)).astype(np.float64)


def _slot_tables():
    """slot s in [0,2048): s<1024 -> Re[k=s]; s==1024 -> Re[1024] (parked in
    Im[0]'s slot, since Im[0] is identically 0); s>1024 -> Im[k=s-1024]."""
    k_of_slot = np.zeros(2048, np.int64)
    is_im = np.zeros(2048, np.bool_)
    for s in range(2048):
        if s < 1024:
            k_of_slot[s] = s
        elif s == 1024:
            k_of_slot[s] = 1024
        else:
            k_of_slot[s] = s - 1024
            is_im[s] = True
    return k_of_slot, is_im


def build_fwd_weights():
    """[2048 n, 2048 slots]: windowed rfft of one frame, slot layout."""
    n = np.arange(WINDOW, dtype=np.float64)
    w = _hann(WINDOW)
    k_of_slot, is_im = _slot_tables()
    ang = 2.0 * np.pi * np.outer(n, k_of_slot.astype(np.float64)) / WINDOW
    W = np.where(is_im[None, :], -np.sin(ang), np.cos(ang))
    W *= w[:, None]
    return W


def build_inv_weights(gain):
    """[2048 slots, 2048 n]: gain * hann * irfft from slot layout."""
    n = np.arange(WINDOW, dtype=np.float64)
    w = _hann(WINDOW)
    k_of_slot, is_im = _slot_tables()
    ang = 2.0 * np.pi * np.outer(k_of_slot.astype(np.float64), n) / WINDOW
    k = k_of_slot
    re_coef = (2.0 - (k == 0) - (k == 1024))[:, None] / WINDOW * np.cos(ang)
    im_coef = -2.0 / WINDOW * np.sin(ang)
    W = np.where(is_im[:, None], im_coef, re_coef)
    W[1024, :] = np.cos(np.pi * n) / WINDOW
    W *= (gain * w)[None, :]
    return W


def _slot_tables_r4():
    """family-major slot layout: slot' = s*512 + local; family s holds
    k = s, s+4, ... <= 1024(ish), (re, im) interleaved k-major."""
    karr = np.zeros(2048, np.int64)
    isim = np.zeros(2048, np.bool_)
    pos = 0
    for s in range(4):
        for k in range(s, 1025, 4):
            karr[pos] = k; isim[pos] = False; pos += 1
            if k not in (0, 1024):
                karr[pos] = k; isim[pos] = True; pos += 1
    assert pos == 2048
    return karr, isim


_R4_PLANES = {0: [0], 1: [2, 3], 2: [1], 3: [2, 3]}  # m//4 -> plane list


def _build_wfam():
    """family -> list of (plane, [512 q, 512 r]) weight matrices.
    planes: 0=y0, 1=y2, 2=m0, 3=m1."""
    q = np.arange(512, dtype=np.float64)
    karr, isim = _slot_tables_r4()
    fams = {}
    for s in range(4):
        kv = karr[s * 512:(s + 1) * 512].astype(np.float64)
        iv = isim[s * 512:(s + 1) * 512]
        ang = 2.0 * np.pi * np.outer(q, kv) / WINDOW
        c, sn = np.cos(ang), np.sin(ang)
        if s == 0:
            fams[s] = [(0, np.where(iv[None, :], -sn, c))]
        elif s == 2:
            fams[s] = [(1, np.where(iv[None, :], -sn, c))]
        elif s == 1:
            fams[s] = [(2, np.where(iv[None, :], -sn, c)),
                       (3, np.where(iv[None, :], -c, -sn))]
        else:
            fams[s] = [(2, np.where(iv[None, :], -sn, c)),
                       (3, np.where(iv[None, :], c, sn))]
    return fams


def build_wf4():
    """[128, 96*128] SBUF-ready block layout matching the device MM loop:
    for qp, mi: m=2qp+mi -> (s=m//4, subm=m%4): for plane, for qc: block
    = Wfam[s][plane][qc*128:+128, subm*128:+128]."""
    fams = _build_wfam()
    blocks = []
    for qp in range(8):
        for mi in range(2):
            m = qp * 2 + mi
            s, subm = m // 4, m % 4
            for pl, Wm in fams[s]:
                for qc in range(4):
                    blocks.append(Wm[qc * 128:(qc + 1) * 128,
                                     subm * 128:(subm + 1) * 128])
    return np.concatenate(blocks, axis=1)  # [128, 96*128]


def build_wcol():
    """[128, 16] per-partition window scalars: col j*4+qc = w[qc*128+p+512j]."""
    w = _hann(WINDOW)
    out = np.zeros((128, 16), np.float64)
    for j in range(4):
        for qc in range(4):
            out[:, j * 4 + qc] = w[qc * 128 + np.arange(128) + 512 * j]
    return out


def build_t_slots(transfer):
    karr, _ = _slot_tables_r4()
    return np.asarray(transfer, np.float64)[:, karr]  # [ch, 2048]


_INV_PLANES = [(0, False), (1, False), (1, True), (2, False), (3, False), (3, True)]
# zb plane order: 0=zre0, 1=zre1, 2=zim1, 3=zre2, 4=zre3, 5=zim3


def build_wi4():
    """[128, 96*128] inverse z-plane weights; device order:
    for qc in 4: for pl in 6: for subm in 4."""
    karr, isim = _slot_tables_r4()
    q = np.arange(512, dtype=np.float64)
    Vs = []
    for (s, want_im) in _INV_PLANES:
        kv = karr[s * 512:(s + 1) * 512].astype(np.float64)
        iv = isim[s * 512:(s + 1) * 512]
        coef = (2.0 - (kv == 0) - (kv == 1024)) / WINDOW
        ang = 2.0 * np.pi * np.outer(kv, q) / WINDOW
        c, sn = np.cos(ang), np.sin(ang)
        V = coef[:, None] * (np.where(iv[:, None], c, sn) if want_im
                             else np.where(iv[:, None], -sn, c))
        Vs.append(V)  # [512 slot-reals, 512 q]
    blocks = []
    for qc in range(4):
        for V in Vs:
            for subm in range(4):
                blocks.append(V[subm * 128:(subm + 1) * 128,
                               qc * 128:(qc + 1) * 128])
    return np.concatenate(blocks, axis=1)


def build_inv_weights_perm(gain):
    """[2048 slots', 2048 n]: gain * hann * irfft from the r4 slot layout."""
    n = np.arange(WINDOW, dtype=np.float64)
    w = _hann(WINDOW)
    karr, isim = _slot_tables_r4()
    k = karr.astype(np.float64)
    ang = 2.0 * np.pi * np.outer(k, n) / WINDOW
    re_coef = (2.0 - (karr == 0) - (karr == 1024))[:, None] / WINDOW * np.cos(ang)
    im_coef = -2.0 / WINDOW * np.sin(ang)
    W = np.where(isim[:, None], im_coef, re_coef)
    W *= (gain * w)[None, :]
    return W


def build_fwd_weights_perm():
    """effective [2048 n, 2048 slots'] fwd matrix (validation only)."""
    n = np.arange(WINDOW, dtype=np.float64)
    w = _hann(WINDOW)
    karr, isim = _slot_tables_r4()
    ang = 2.0 * np.pi * np.outer(n, karr.astype(np.float64)) / WINDOW
    W = np.where(isim[None, :], -np.sin(ang), np.cos(ang))
    W *= w[:, None]
    return W


def build_pattern(t_slots_core):
    """T-pattern [128, SPECW]: per (m,d) chain block of CB cols:
    col 0 = 0 (reset/inject), cols 1..16 = T[slot(m,kf), d]."""
    pat = np.zeros((128, SPECW), np.float64)
    for m in range(NM):
        for d in range(DSH):
            base = (m * DSH + d) * CB
            pat[:, base + 1: base + CB] = \
                t_slots_core[d, m * 128:(m + 1) * 128][:, None]
    return pat


def emulate(x, transfer, mixer_matrix, gain, wdtype=np.float32):
    """Numpy emulation of the device math (offline validation)."""
    b, c, t = x.shape
    Wf = build_fwd_weights_perm().astype(wdtype).astype(np.float64)
    Wi = build_inv_weights_perm(float(np.asarray(gain).ravel()[0])).astype(wdtype).astype(np.float64)
    Ts = build_t_slots(transfer)
    y = np.einsum('bct,cd->bdt', np.asarray(x, np.float64),
                  np.asarray(mixer_matrix, np.float64))
    yp = np.pad(y, ((0, 0), (0, 0), (0, STEP)))
    out = np.zeros((b, c, t), np.float64)
    for bi in range(b):
        frames = np.stack([yp[bi, :, f * STEP: f * STEP + WINDOW]
                           for f in range(FRAMES)], 1)
        spec = frames.astype(wdtype).astype(np.float64) @ Wf
        st = np.zeros((c, 2048))
        outs = np.zeros_like(spec)
        for f in range(FRAMES):
            st = (spec[:, f].astype(wdtype).astype(np.float64) + st) * Ts
            outs[:, f] = st
        aud = outs.astype(wdtype).astype(np.float64) @ Wi
        acc = np.zeros((c, t + STEP))
        for f in range(FRAMES):
            acc[:, f * STEP: f * STEP + WINDOW] += aud[:, f]
        out[bi] = np.tanh(acc[:, :t])
    return out.astype(np.float32)


# ---------------------------------------------------------------------------
# Device program
# ---------------------------------------------------------------------------

_CACHED_NC = None


def _build_program():
    import concourse.bacc as bacc
    import concourse.mybir as mybir
    from concourse import tile
    from contextlib import ExitStack

    f32 = mybir.dt.float32
    bf16 = mybir.dt.bfloat16
    Alu = mybir.AluOpType

    nc = bacc.Bacc("TRN2", target_bir_lowering=False, debug=False, num_devices=8)
    xb = nc.dram_tensor("xb", [CPD, TIME], bf16, kind="ExternalInput").ap()
    mixw = nc.dram_tensor("mixw", [CPD, DSH], bf16, kind="ExternalInput").ap()
    wf4 = nc.dram_tensor("wf4", [128, 96 * 128], bf16, kind="ExternalInput").ap()
    wcold = nc.dram_tensor("wcol", [128, 16], f32, kind="ExternalInput").ap()
    wi4d = nc.dram_tensor("wi4", [128, 96 * 128], bf16, kind="ExternalInput").ap()
    wicold = nc.dram_tensor("wicol", [128, 16], f32, kind="ExternalInput").ap()
    patd = nc.dram_tensor("pat", [128, SPECW], bf16, kind="ExternalInput").ap()
    eyed = nc.dram_tensor("eye", [128, 128], f32, kind="ExternalInput").ap()
    eyebd = nc.dram_tensor("eyeb", [128, 128], bf16, kind="ExternalInput").ap()
    yout = nc.dram_tensor("y", [DSH, TIME], f32, kind="ExternalOutput").ap()

    XCH = 2048           # x streamed in [64, 2048] chunks (16 g-chunks each)
    NXC = TIME // XCH    # 32

    with tile.TileContext(nc) as tc, ExitStack() as ctx:
        persist = ctx.enter_context(tc.tile_pool(name="persist", bufs=1))
        spec = persist.tile([128, FC * SPECW], bf16, tag="spec")
        pat = persist.tile([128, SPECW], bf16, tag="pat")
        mx = persist.tile([CPD, DSH], bf16, tag="mx")
        eyeb = persist.tile([128, 128], bf16, tag="eyeb")
        eye = persist.tile([128, 128], f32, tag="eye")
        wcol = persist.tile([128, 16], f32, tag="wcol")

        # PSUM (8 banks of 2KB/partition):
        #   ppA [128,512] f32 x2 = 2 banks  (mixer psum, then phase-I OLA)
        #   ppB [128,512] f32 x2 = 2 banks  (corner-turn psum, then phase-I t4)
        #   sp  [128,1024] f32 x2 = 4 banks (fwd DFT accumulators)
        ppA = ctx.enter_context(tc.tile_pool(name="ppA", bufs=3, space="PSUM"))
        ppB = ctx.enter_context(tc.tile_pool(name="ppB", bufs=3, space="PSUM"))

        xin = ctx.enter_context(tc.tile_pool(name="xin", bufs=2))
        ymp = ctx.enter_context(tc.tile_pool(name="ymp", bufs=2))

        # small/early tensors on the sync queue; big weights on gpsimd queue
        nc.sync.dma_start(out=mx[:], in_=mixw[:])
        nc.sync.dma_start(out=eyeb[:], in_=eyebd[:])
        nc.sync.dma_start(out=wcol[:], in_=wcold[:])
        nc.sync.dma_start(out=eye[:], in_=eyed[:])

        # chain col 0 of the first fc block must read as 0 (fresh state)
        nc.vector.memset(
            spec[:][:, 0:SPECW].rearrange(
                "p (md c) -> p md c", c=CB)[:, :, 0:1], 0.0)

        def mixer_chunk(xc):
            xt = xin.tile([CPD, XCH], bf16, tag="x", name=f"x{xc}")
            nc.sync.dma_start(out=xt[:], in_=xb[:, xc * XCH:(xc + 1) * XCH])
            pm = ppA.tile([128, 512], f32, tag="pp", name=f"mix{xc}")
            for q in range(4):
                nc.tensor.matmul(
                    pm[q * DSH:(q + 1) * DSH, :],
                    mx[:],
                    xt[:, q * 512:(q + 1) * 512],
                    start=True, stop=True,
                    tile_position=(0, q * DSH))
            ym = ymp.tile([128, 512], bf16, tag="ym", name=f"ym{xc}")
            nc.scalar.copy(ym[:], pm[:])
            # ym[(q,d), tloc]: t = xc*2048 + q*512 + tloc
            for gq in range(4):  # per 4 g-chunks (one psum turn tile)
                pt = ppB.tile([128, 128], bf16, tag="pp", name=f"turn{xc}_{gq}")
                nc.tensor.transpose(
                    pt[:],
                    ym[:, gq * 128: gq * 128 + 128],
                    eyeb[:])
                # pt[tfine, (q2, d)] covers g = xc*16 + q2*4 + gq
                g0 = xc * (XCH // 128)
                dst = a_t[:][:, g0 * DSH:(g0 + 16) * DSH] \
                    .rearrange("p (q2 gq d) -> p q2 gq d", q2=4, gq=4)[
                        :, :, gq, :]
                psrc = pt[:].rearrange("p (q2 d) -> p q2 d", q2=4)
                nc.scalar.copy(dst, psrc)

        def precombine(b, wf_t, ztp, xwp, tmpp):
            """butterfly planes for frame batch b: zt cols (plane, qc, f, d);
            planes 0=y0, 1=y2, 2=m0, 3=m1.  Window applied via per-partition
            tensor_scalar on GpSimd; adds on DVE."""
            zt = ztp.tile([128, 4 * 4 * 512], bf16, tag="zt", name=f"zt{b}")
            for qc in range(4):
                xw = xwp.tile([128, 2048], bf16, tag="xw", name=f"xw{b}_{qc}")
                for j in range(4):
                    base = (128 * b + qc + 4 * j) * DSH
                    view = a_t[:][:, base: base + 4096] \
                        .rearrange("p (f q) -> p f q", f=16)[:, :, :DSH]
                    nc.gpsimd.tensor_scalar_mul(
                        xw[:, j * 512:(j + 1) * 512]
                        .rearrange("p (f d) -> p f d", f=16),
                        view, wcol[:, j * 4 + qc: j * 4 + qc + 1])
                tmp = tmpp.tile([128, 1024], bf16, tag="tmp", name=f"tm{b}_{qc}")
                eng = nc.vector if qc % 2 == 0 else nc.gpsimd
                eng.tensor_add(tmp[:, :512], xw[:, 0:512], xw[:, 1024:1536])
                eng.tensor_add(tmp[:, 512:], xw[:, 512:1024], xw[:, 1536:2048])
                z = lambda pl: zt[:, (pl * 4 + qc) * 512:(pl * 4 + qc + 1) * 512]
                eng.tensor_sub(z(2), xw[:, 0:512], xw[:, 1024:1536])
                eng.tensor_sub(z(3), xw[:, 512:1024], xw[:, 1536:2048])
                eng.tensor_add(z(0), tmp[:, :512], tmp[:, 512:])
                eng.tensor_sub(z(1), tmp[:, :512], tmp[:, 512:])
            return zt

        def fwd_batch(f16, wf_t, zt):
            # radix-4 forward: per m-block, accumulate plane x qc matmuls
            fc = f16
            blk = [0]
            for m in range(16):
                ps = sp.tile([128, 512], f32, tag="sm", name=f"sm{f16}_{m}")
                s = m // 4
                planes = _R4_PLANES[s]
                out_ap = ps[:].rearrange("p (d f) -> p f d", f=16)
                nmm = len(planes) * 4
                i = 0
                for pl in planes:
                    for qc in range(4):
                        rhs = zt[:, (pl * 4 + qc) * 512:(pl * 4 + qc + 1) * 512] \
                            .rearrange("p (f d) -> p f d", f=16)
                        nc.tensor.matmul(
                            out_ap,
                            wf_t[:, blk[0] * 128:(blk[0] + 1) * 128],
                            rhs,
                            start=(i == 0), stop=(i == nmm - 1))
                        blk[0] += 1
                        i += 1
                # per-m eviction (ScalarE; DVE is scanning)
                src_ = ps[:].rearrange("p (d f) -> p d f", f=16)
                doff = fc * SPECW + m * DSH * CB
                dst = spec[:][:, doff: doff + DSH * CB] \
                    .rearrange("p (d c) -> p d c", c=CB)[:, :, 1: 1 + FW]
                nc.scalar.copy(dst, src_)

        def scan_block(fc):
            # recurrence scan for frames [16fc, 16fc+16); inject copy to the
            # next block must happen BEFORE the in-place T*u multiply.
            nc.vector.tensor_tensor_scan(
                spec[:, fc * SPECW:(fc + 1) * SPECW],
                pat[:],
                spec[:, fc * SPECW:(fc + 1) * SPECW],
                0.0, Alu.mult, Alu.add)
            if fc + 1 < FC:
                src = spec[:][:, fc * SPECW: (fc + 1) * SPECW] \
                    .rearrange("p (md c) -> p md c", c=CB)[:, :, CB - 1: CB]
                dst = spec[:][:, (fc + 1) * SPECW: (fc + 2) * SPECW] \
                    .rearrange("p (md c) -> p md c", c=CB)[:, :, 0:1]
                nc.vector.tensor_copy(dst, src)
            nc.vector.tensor_mul(
                spec[:, fc * SPECW:(fc + 1) * SPECW],
                spec[:, fc * SPECW:(fc + 1) * SPECW],
                pat[:])

        # ================= phase F (+ scan), pipelined =================
        with ExitStack() as ctxF:
            wp = ctxF.enter_context(tc.tile_pool(name="wfp", bufs=1))
            sp = ctxF.enter_context(tc.tile_pool(name="sp", bufs=2, space="PSUM"))
            wf_t = wp.tile([128, 96 * 128], bf16, tag="wf")
            a_t = wp.tile([128, GPAD * DSH], bf16, tag="a")
            nc.vector.memset(a_t[:, GCH * DSH:], 0.0)
            ztp = ctxF.enter_context(tc.tile_pool(name="ztp", bufs=2))
            xwp = ctxF.enter_context(tc.tile_pool(name="xwp", bufs=2))
            tmpp = ctxF.enter_context(tc.tile_pool(name="tmpp", bufs=2))

            for xc in range(9):
                mixer_chunk(xc)
            # weights AFTER the first x chunks on the queue: mixer starts
            # immediately, wf4/pat still arrive before first use
            nc.sync.dma_start(out=wf_t[:], in_=wf4[:])
            nc.sync.dma_start(out=pat[:], in_=patd[:])
            zt0 = precombine(0, wf_t, ztp, xwp, tmpp)
            for xc in range(9, 17):
                mixer_chunk(xc)
            zt1 = precombine(1, wf_t, ztp, xwp, tmpp)
            fwd_batch(0, wf_t, zt0)
            for xc in range(17, 25):
                mixer_chunk(xc)
            scan_block(0)
            zt2 = precombine(2, wf_t, ztp, xwp, tmpp)
            fwd_batch(1, wf_t, zt1)
            for xc in range(25, NXC):
                mixer_chunk(xc)
            scan_block(1)
            zt3 = precombine(3, wf_t, ztp, xwp, tmpp)
            fwd_batch(2, wf_t, zt2)
            scan_block(2)
            fwd_batch(3, wf_t, zt3)
            scan_block(3)

        # ================= phase I (radix-4 inverse) =================
        with ExitStack() as ctxI:
            wp2 = ctxI.enter_context(tc.tile_pool(name="wip", bufs=1))
            wi_t = wp2.tile([128, 96 * 128], bf16, tag="wi")
            nc.sync.dma_start(out=wi_t[:, :48 * 128], in_=wi4d[:, :48 * 128])
            nc.scalar.dma_start(out=wi_t[:, 48 * 128:], in_=wi4d[:, 48 * 128:])
            wicol = wp2.tile([128, 16], f32, tag="wicol")
            nc.sync.dma_start(out=wicol[:], in_=wicold[:])
            ztail = wp2.tile([128, 2 * 4 * DSH], bf16, tag="ztail")
            nc.vector.memset(ztail[:], 0.0)

            tout = ctxI.enter_context(tc.tile_pool(name="tout", bufs=6))
            stg = ctxI.enter_context(tc.tile_pool(name="stg", bufs=3))
            zbp = ctxI.enter_context(tc.tile_pool(name="zbp", bufs=4))
            efp = ctxI.enter_context(tc.tile_pool(name="efp", bufs=3))
            aqp = ctxI.enter_context(tc.tile_pool(name="aqp", bufs=3))
            ywp = ctxI.enter_context(tc.tile_pool(name="ywp", bufs=3))
            ohp = ctxI.enter_context(tc.tile_pool(name="ohp", bufs=3))
            tailp = ctxI.enter_context(tc.tile_pool(name="tailp", bufs=2))

            yv = yout.rearrange("d (a4 fl t) -> fl d a4 t", fl=4, t=1024)

            def emit_store(tt, fc, qc, h):
                p4 = ppB.tile([128, 512], f32, tag="pp",
                              name=f"t4_{fc}_{qc}_{h}")
                for r2 in range(4):
                    nc.tensor.transpose(
                        p4[:, r2 * 128:(r2 + 1) * 128],
                        tt[:, r2 * 128:(r2 + 1) * 128],
                        eye[:])
                st = stg.tile([128, 512], f32, tag="stg",
                              name=f"stg{fc}_{qc}_{h}")
                if h == 0:
                    nc.vector.tensor_copy(st[:], p4[:])
                else:
                    nc.scalar.copy(st[:], p4[:])
                deng = nc.sync if h == 0 else nc.scalar
                for r2 in range(4):
                    dst = yv[:, :, 4 * fc + r2,
                             512 * h + 128 * qc: 512 * h + 128 * qc + 128]
                    deng.dma_start(
                        out=dst,
                        in_=st[:, r2 * 128:(r2 + 1) * 128])

            deferred = []
            tail_prev = ztail
            for fc in range(FC):
                tail_new = tailp.tile([128, 2 * 4 * DSH], bf16, tag="tail",
                                      name=f"tail{fc}") if fc < FC - 1 else None
                for qc in range(4):
                    # 6 z-plane transforms: contraction over family slot-reals
                    zb = zbp.tile([128, 6 * 512], bf16, tag="zb",
                                  name=f"zb{fc}_{qc}")
                    for pl in range(6):
                        s = _INV_PLANES[pl][0]
                        ps = ppA.tile([128, 512], f32, tag="pp",
                                      name=f"zp{fc}_{qc}_{pl}")
                        out_ap = ps[:].rearrange("p (f d) -> p d f", f=FW)
                        for subm in range(4):
                            m = 4 * s + subm
                            base = fc * SPECW + m * DSH * CB
                            rhs = spec[:][:, base: base + DSH * CB] \
                                .rearrange("p (d c) -> p d c", c=CB)[:, :, 1: 1 + FW]
                            blk = (qc * 6 + pl) * 4 + subm
                            nc.tensor.matmul(
                                out_ap,
                                wi_t[:, blk * 128:(blk + 1) * 128],
                                rhs, start=(subm == 0), stop=(subm == 3))
                        nc.scalar.copy(zb[:, pl * 512:(pl + 1) * 512], ps[:])
                    # butterflies (DVE, bf16): e,f,gg,h then quarters a0..a3
                    ef = efp.tile([128, 4 * 512], bf16, tag="ef",
                                  name=f"ef{fc}_{qc}")
                    z = lambda pl: zb[:, pl * 512:(pl + 1) * 512]
                    nc.vector.tensor_add(ef[:, 0 * 512:1 * 512], z(0), z(3))   # e
                    nc.vector.tensor_sub(ef[:, 1 * 512:2 * 512], z(0), z(3))   # f
                    nc.vector.tensor_add(ef[:, 2 * 512:3 * 512], z(1), z(4))   # gg
                    nc.vector.tensor_sub(ef[:, 3 * 512:4 * 512], z(5), z(2))   # h
                    aq = aqp.tile([128, 4 * 512], bf16, tag="aq",
                                  name=f"aq{fc}_{qc}")
                    E, F_, G, H = (ef[:, i * 512:(i + 1) * 512] for i in range(4))
                    nc.vector.tensor_add(aq[:, 0 * 512:1 * 512], E, G)   # a0
                    nc.vector.tensor_add(aq[:, 1 * 512:2 * 512], F_, H)  # a1
                    nc.vector.tensor_sub(aq[:, 2 * 512:3 * 512], E, G)   # a2
                    nc.vector.tensor_sub(aq[:, 3 * 512:4 * 512], F_, H)  # a3
                    # save pre-window tail quarters (a2,a3 of frame 15)
                    if tail_new is not None:
                        for j2 in range(2):
                            nc.vector.tensor_copy(
                                tail_new[:, (j2 * 4 + qc) * DSH:
                                         (j2 * 4 + qc + 1) * DSH],
                                aq[:, (2 + j2) * 512 + 15 * DSH:
                                   (2 + j2) * 512 + 16 * DSH])
                    # window (GpSimd, per-partition scalars) + OLA + tanh
                    for h in range(2):
                        yw = ywp.tile([128, 1024], f32, tag="yw",
                                      name=f"yw{fc}_{qc}_{h}")
                        nc.gpsimd.tensor_scalar_mul(
                            yw[:, :512], aq[:, h * 512:(h + 1) * 512],
                            wicol[:, h * 4 + qc: h * 4 + qc + 1])
                        nc.gpsimd.tensor_scalar_mul(
                            yw[:, 512:], aq[:, (h + 2) * 512:(h + 3) * 512],
                            wicol[:, (h + 2) * 4 + qc: (h + 2) * 4 + qc + 1])
                        # windowed tail quarter for frame 0 of this batch
                        wt = ywp.tile([128, DSH], f32, tag="wt",
                                      name=f"wt{fc}_{qc}_{h}")
                        nc.gpsimd.tensor_scalar_mul(
                            wt[:], tail_prev[:, (h * 4 + qc) * DSH:
                                             (h * 4 + qc + 1) * DSH],
                            wicol[:, (h + 2) * 4 + qc: (h + 2) * 4 + qc + 1])
                        oh = ohp.tile([128, 512], f32, tag="oh",
                                      name=f"oh{fc}_{qc}_{h}")
                        nc.vector.tensor_add(
                            oh[:, DSH:], yw[:, DSH:512], yw[:, 512:1024 - DSH])
                        nc.vector.tensor_add(oh[:, :DSH], yw[:, :DSH], wt[:])
                        # tanh now; corner-turn/store deferred 2 qc-groups
                        tt = tout.tile([128, 512], f32, tag="to",
                                       name=f"to{fc}_{qc}_{h}")
                        nc.scalar.activation(
                            tt[:], oh[:], mybir.ActivationFunctionType.Tanh)
                        deferred.append((tt, fc, qc, h))
                    while len(deferred) > 4:
                        emit_store(*deferred.pop(0))
                tail_prev = tail_new if tail_new is not None else ztail
            while deferred:
                emit_store(*deferred.pop(0))
    nc.compile()
    return nc


def _get_nc():
    global _CACHED_NC
    if _CACHED_NC is None:
        _CACHED_NC = _build_program()
    return _CACHED_NC


def kernel(x, transfer, mixer_matrix, gain, _trace=False):
    import ml_dtypes
    from concourse.bass_utils import run_bass_kernel_spmd

    x = np.ascontiguousarray(np.asarray(x, np.float32))
    transfer = np.asarray(transfer, np.float32)
    mixer_matrix = np.asarray(mixer_matrix, np.float32)
    gain = np.asarray(gain, np.float32)

    bf = ml_dtypes.bfloat16
    wf4_np = build_wf4().astype(bf)
    wcol_np = build_wcol().astype(np.float32)
    wi4_np = build_wi4().astype(bf)
    wicol_np = (float(gain.ravel()[0]) * build_wcol()).astype(np.float32)
    Ts = build_t_slots(transfer)
    eye = np.eye(128, dtype=np.float32)
    eyeb = np.eye(128, dtype=np.float64).astype(bf)

    in_maps = []
    for c in range(8):
        b, dh = c // 2, c % 2
        mixw = mixer_matrix[:, dh * DSH:(dh + 1) * DSH].astype(bf)
        patc = build_pattern(Ts[dh * DSH:(dh + 1) * DSH]).astype(bf)
        in_maps.append({
            "xb": x[b].astype(bf),
            "mixw": mixw,
            "wf4": wf4_np,
            "wcol": wcol_np,
            "wi4": wi4_np,
            "wicol": wicol_np,
            "pat": patc,
            "eye": eye,
            "eyeb": eyeb,
        })

    nc = _get_nc()
    res = run_bass_kernel_spmd(nc, in_maps, list(range(8)), trace=_trace)
    out = np.zeros((BATCH, CPD, TIME), np.float32)
    for c in range(8):
        b, dh = c // 2, c % 2
        out[b, dh * DSH:(dh + 1) * DSH] = res.results[c]["y"]
    if _trace:
        return out, res
    return out


# revision 21
# speedup vs baseline: 1.1395x; 1.1395x over previous
"""Trainium2 Bass kernel for nn_Block_38517266710836.

reference pipeline: channel mixer -> STFT (hann 2048, hop 1024) -> per-frame
recurrence out[f] = (spec[f] + out[f-1]) * transfer -> iSTFT (hann synthesis)
-> overlap-add -> gain -> tanh.

Sharding: 8 cores, data-parallel over (batch, channel-half): core c handles
batch c//2, mixed channels [32*(c%2), +32). Each core receives its batch's
full 64-channel input (the mixer contracts channels) and writes 32 rows.

Pipelined single-pass program per core: mixer chunks, forward-DFT frame
batches, the DVE recurrence scan, and the inverse/overlap-add phase are
interleaved so the scan and evictions hide under PE matmul work.  Forward
evictions and corner-turn copies run on ScalarE (DVE is reserved for the
scan), weights stream on the gpsimd DMA queue, x/y on the sync queue, and
PSUM pools are shared across phases to fit the 8-bank budget.
"""

import numpy as np

WINDOW = 2048
STEP = 1024
CPD = 64
BATCH = 4
TIME = 65536
FRAMES = 64
NJ = 16              # per-frame time chunks (fwd contraction blocks)
NM = 16              # spectral slot chunks
DSH = 32             # mixed channels per core
GCH = TIME // 128    # 512 global 128-sample chunks
GPAD = GCH + 16      # + zero pad (frame 63 reaches t=66560; extra width so
                     # the forward rhs slice [base, base+2048) stays in-bounds)
FC = 4               # frame chunks for the scan layout
FW = 16              # frames per chunk
CB = 17              # chain block: 1 inject/reset col + 16 frame cols
SPECW = NM * DSH * CB  # 8704 free cols per fc block


def _hann(n):
    return (0.5 - 0.5 * np.cos(2.0 * np.pi * np.arange(n) / n)).astype(np.float64)


def _slot_tables():
    """slot s in [0,2048): s<1024 -> Re[k=s]; s==1024 -> Re[1024] (parked in
    Im[0]'s slot, since Im[0] is identically 0); s>1024 -> Im[k=s-1024]."""
    k_of_slot = np.zeros(2048, np.int64)
    is_im = np.zeros(2048, np.bool_)
    for s in range(2048):
        if s < 1024:
            k_of_slot[s] = s
        elif s == 1024:
            k_of_slot[s] = 1024
        else:
            k_of_slot[s] = s - 1024
            is_im[s] = True
    return k_of_slot, is_im


def build_fwd_weights():
    """[2048 n, 2048 slots]: windowed rfft of one frame, slot layout."""
    n = np.arange(WINDOW, dtype=np.float64)
    w = _hann(WINDOW)
    k_of_slot, is_im = _slot_tables()
    ang = 2.0 * np.pi * np.outer(n, k_of_slot.astype(np.float64)) / WINDOW
    W = np.where(is_im[None, :], -np.sin(ang), np.cos(ang))
    W *= w[:, None]
    return W


def build_inv_weights(gain):
    """[2048 slots, 2048 n]: gain * hann * irfft from slot layout."""
    n = np.arange(WINDOW, dtype=np.float64)
    w = _hann(WINDOW)
    k_of_slot, is_im = _slot_tables()
    ang = 2.0 * np.pi * np.outer(k_of_slot.astype(np.float64), n) / WINDOW
    k = k_of_slot
    re_coef = (2.0 - (k == 0) - (k == 1024))[:, None] / WINDOW * np.cos(ang)
    im_coef = -2.0 / WINDOW * np.sin(ang)
    W = np.where(is_im[:, None], im_coef, re_coef)
    W[1024, :] = np.cos(np.pi * n) / WINDOW
    W *= (gain * w)[None, :]
    return W


def _slot_tables_r4():
    """family-major slot layout: slot' = s*512 + local; family s holds
    k = s, s+4, ... <= 1024(ish), (re, im) interleaved k-major."""
    karr = np.zeros(2048, np.int64)
    isim = np.zeros(2048, np.bool_)
    pos = 0
    for s in range(4):
        for k in range(s, 1025, 4):
            karr[pos] = k; isim[pos] = False; pos += 1
            if k not in (0, 1024):
                karr[pos] = k; isim[pos] = True; pos += 1
    assert pos == 2048
    return karr, isim


_R4_PLANES = {0: [0], 1: [2, 3], 2: [1], 3: [2, 3]}  # m//4 -> plane list


def _build_wfam():
    """family -> list of (plane, [512 q, 512 r]) weight matrices.
    planes: 0=y0, 1=y2, 2=m0, 3=m1."""
    q = np.arange(512, dtype=np.float64)
    karr, isim = _slot_tables_r4()
    fams = {}
    for s in range(4):
        kv = karr[s * 512:(s + 1) * 512].astype(np.float64)
        iv = isim[s * 512:(s + 1) * 512]
        ang = 2.0 * np.pi * np.outer(q, kv) / WINDOW
        c, sn = np.cos(ang), np.sin(ang)
        if s == 0:
            fams[s] = [(0, np.where(iv[None, :], -sn, c))]
        elif s == 2:
            fams[s] = [(1, np.where(iv[None, :], -sn, c))]
        elif s == 1:
            fams[s] = [(2, np.where(iv[None, :], -sn, c)),
                       (3, np.where(iv[None, :], -c, -sn))]
        else:
            fams[s] = [(2, np.where(iv[None, :], -sn, c)),
                       (3, np.where(iv[None, :], c, sn))]
    return fams


def build_wf4():
    """[128, 96*128] SBUF-ready block layout matching the device MM loop:
    for qp, mi: m=2qp+mi -> (s=m//4, subm=m%4): for plane, for qc: block
    = Wfam[s][plane][qc*128:+128, subm*128:+128]."""
    fams = _build_wfam()
    blocks = []
    for qp in range(8):
        for mi in range(2):
            m = qp * 2 + mi
            s, subm = m // 4, m % 4
            for pl, Wm in fams[s]:
                for qc in range(4):
                    blocks.append(Wm[qc * 128:(qc + 1) * 128,
                                     subm * 128:(subm + 1) * 128])
    return np.concatenate(blocks, axis=1)  # [128, 96*128]


def build_wcol():
    """[128, 16] per-partition window scalars: col j*4+qc = w[qc*128+p+512j]."""
    w = _hann(WINDOW)
    out = np.zeros((128, 16), np.float64)
    for j in range(4):
        for qc in range(4):
            out[:, j * 4 + qc] = w[qc * 128 + np.arange(128) + 512 * j]
    return out


def build_t_slots(transfer):
    karr, _ = _slot_tables_r4()
    return np.asarray(transfer, np.float64)[:, karr]  # [ch, 2048]


_INV_PLANES = [(0, False), (1, False), (1, True), (2, False), (3, False), (3, True)]
# zb plane order: 0=zre0, 1=zre1, 2=zim1, 3=zre2, 4=zre3, 5=zim3


def build_wi4():
    """[128, 96*128] inverse z-plane weights; device order:
    for qc in 4: for pl in 6: for subm in 4."""
    karr, isim = _slot_tables_r4()
    q = np.arange(512, dtype=np.float64)
    Vs = []
    for (s, want_im) in _INV_PLANES:
        kv = karr[s * 512:(s + 1) * 512].astype(np.float64)
        iv = isim[s * 512:(s + 1) * 512]
        coef = (2.0 - (kv == 0) - (kv == 1024)) / WINDOW
        ang = 2.0 * np.pi * np.outer(kv, q) / WINDOW
        c, sn = np.cos(ang), np.sin(ang)
        V = coef[:, None] * (np.where(iv[:, None], c, sn) if want_im
                             else np.where(iv[:, None], -sn, c))
        Vs.append(V)  # [512 slot-reals, 512 q]
    blocks = []
    for qc in range(4):
        for V in Vs:
            for subm in range(4):
                blocks.append(V[subm * 128:(subm + 1) * 128,
                               qc * 128:(qc + 1) * 128])
    return np.concatenate(blocks, axis=1)


def build_inv_weights_perm(gain):
    """[2048 slots', 2048 n]: gain * hann * irfft from the r4 slot layout."""
    n = np.arange(WINDOW, dtype=np.float64)
    w = _hann(WINDOW)
    karr, isim = _slot_tables_r4()
    k = karr.astype(np.float64)
    ang = 2.0 * np.pi * np.outer(k, n) / WINDOW
    re_coef = (2.0 - (karr == 0) - (karr == 1024))[:, None] / WINDOW * np.cos(ang)
    im_coef = -2.0 / WINDOW * np.sin(ang)
    W = np.where(isim[:, None], im_coef, re_coef)
    W *= (gain * w)[None, :]
    return W


def build_fwd_weights_perm():
    """effective [2048 n, 2048 slots'] fwd matrix (validation only)."""
    n = np.arange(WINDOW, dtype=np.float64)
    w = _hann(WINDOW)
    karr, isim = _slot_tables_r4()
    ang = 2.0 * np.pi * np.outer(n, karr.astype(np.float64)) / WINDOW
    W = np.where(isim[None, :], -np.sin(ang), np.cos(ang))
    W *= w[:, None]
    return W


def build_pattern(t_slots_core):
    """T-pattern [128, SPECW]: per (m,d) chain block of CB cols:
    col 0 = 0 (reset/inject), cols 1..16 = T[slot(m,kf), d]."""
    pat = np.zeros((128, SPECW), np.float64)
    for m in range(NM):
        for d in range(DSH):
            base = (m * DSH + d) * CB
            pat[:, base + 1: base + CB] = \
                t_slots_core[d, m * 128:(m + 1) * 128][:, None]
    return pat


def emulate(x, transfer, mixer_matrix, gain, wdtype=np.float32):
    """Numpy emulation of the device math (offline validation)."""
    b, c, t = x.shape
    Wf = build_fwd_weights_perm().astype(wdtype).astype(np.float64)
    Wi = build_inv_weights_perm(float(np.asarray(gain).ravel()[0])).astype(wdtype).astype(np.float64)
    Ts = build_t_slots(transfer)
    y = np.einsum('bct,cd->bdt', np.asarray(x, np.float64),
                  np.asarray(mixer_matrix, np.float64))
    yp = np.pad(y, ((0, 0), (0, 0), (0, STEP)))
    out = np.zeros((b, c, t), np.float64)
    for bi in range(b):
        frames = np.stack([yp[bi, :, f * STEP: f * STEP + WINDOW]
                           for f in range(FRAMES)], 1)
        spec = frames.astype(wdtype).astype(np.float64) @ Wf
        st = np.zeros((c, 2048))
        outs = np.zeros_like(spec)
        for f in range(FRAMES):
            st = (spec[:, f].astype(wdtype).astype(np.float64) + st) * Ts
            outs[:, f] = st
        aud = outs.astype(wdtype).astype(np.float64) @ Wi
        acc = np.zeros((c, t + STEP))
        for f in range(FRAMES):
            acc[:, f * STEP: f * STEP + WINDOW] += aud[:, f]
        out[bi] = np.tanh(acc[:, :t])
    return out.astype(np.float32)


# ---------------------------------------------------------------------------
# Device program
# ---------------------------------------------------------------------------

_CACHED_NC = None


def _build_program():
    import concourse.bacc as bacc
    import concourse.mybir as mybir
    from concourse import tile
    from contextlib import ExitStack

    f32 = mybir.dt.float32
    bf16 = mybir.dt.bfloat16
    Alu = mybir.AluOpType

    nc = bacc.Bacc("TRN2", target_bir_lowering=False, debug=False, num_devices=8)
    xb = nc.dram_tensor("xb", [CPD, TIME], bf16, kind="ExternalInput").ap()
    mixw = nc.dram_tensor("mixw", [CPD, DSH], bf16, kind="ExternalInput").ap()
    wf4 = nc.dram_tensor("wf4", [128, 96 * 128], bf16, kind="ExternalInput").ap()
    wcold = nc.dram_tensor("wcol", [128, 16], f32, kind="ExternalInput").ap()
    wi4d = nc.dram_tensor("wi4", [128, 96 * 128], bf16, kind="ExternalInput").ap()
    wicold = nc.dram_tensor("wicol", [128, 16], f32, kind="ExternalInput").ap()
    patd = nc.dram_tensor("pat", [128, SPECW], bf16, kind="ExternalInput").ap()
    eyed = nc.dram_tensor("eye", [128, 128], f32, kind="ExternalInput").ap()
    eyebd = nc.dram_tensor("eyeb", [128, 128], bf16, kind="ExternalInput").ap()
    yout = nc.dram_tensor("y", [DSH, TIME], f32, kind="ExternalOutput").ap()

    XCH = 2048           # x streamed in [64, 2048] chunks (16 g-chunks each)
    NXC = TIME // XCH    # 32

    with tile.TileContext(nc) as tc, ExitStack() as ctx:
        persist = ctx.enter_context(tc.tile_pool(name="persist", bufs=1))
        spec = persist.tile([128, FC * SPECW], bf16, tag="spec")
        pat = persist.tile([128, SPECW], bf16, tag="pat")
        mx = persist.tile([CPD, DSH], bf16, tag="mx")
        eyeb = persist.tile([128, 128], bf16, tag="eyeb")
        eye = persist.tile([128, 128], f32, tag="eye")
        wcol = persist.tile([128, 16], f32, tag="wcol")

        # PSUM (8 banks of 2KB/partition):
        #   ppA [128,512] f32 x2 = 2 banks  (mixer psum, then phase-I OLA)
        #   ppB [128,512] f32 x2 = 2 banks  (corner-turn psum, then phase-I t4)
        #   sp  [128,1024] f32 x2 = 4 banks (fwd DFT accumulators)
        ppA = ctx.enter_context(tc.tile_pool(name="ppA", bufs=3, space="PSUM"))
        ppB = ctx.enter_context(tc.tile_pool(name="ppB", bufs=3, space="PSUM"))

        xin = ctx.enter_context(tc.tile_pool(name="xin", bufs=2))
        ymp = ctx.enter_context(tc.tile_pool(name="ymp", bufs=2))

        # small/early tensors on the sync queue; big weights on gpsimd queue
        nc.sync.dma_start(out=mx[:], in_=mixw[:])
        nc.sync.dma_start(out=eyeb[:], in_=eyebd[:])
        nc.sync.dma_start(out=wcol[:], in_=wcold[:])
        nc.sync.dma_start(out=eye[:], in_=eyed[:])

        # chain col 0 of the first fc block must read as 0 (fresh state)
        nc.vector.memset(
            spec[:][:, 0:SPECW].rearrange(
                "p (md c) -> p md c", c=CB)[:, :, 0:1], 0.0)

        def mixer_chunk(xc):
            xt = xin.tile([CPD, XCH], bf16, tag="x", name=f"x{xc}")
            nc.sync.dma_start(out=xt[:], in_=xb[:, xc * XCH:(xc + 1) * XCH])
            pm = ppA.tile([128, 512], f32, tag="pp", name=f"mix{xc}")
            for q in range(4):
                nc.tensor.matmul(
                    pm[q * DSH:(q + 1) * DSH, :],
                    mx[:],
                    xt[:, q * 512:(q + 1) * 512],
                    start=True, stop=True,
                    tile_position=(0, q * DSH))
            ym = ymp.tile([128, 512], bf16, tag="ym", name=f"ym{xc}")
            nc.scalar.copy(ym[:], pm[:])
            # ym[(q,d), tloc]: t = xc*2048 + q*512 + tloc
            for gq in range(4):  # per 4 g-chunks (one psum turn tile)
                pt = ppB.tile([128, 128], bf16, tag="pp", name=f"turn{xc}_{gq}")
                nc.tensor.transpose(
                    pt[:],
                    ym[:, gq * 128: gq * 128 + 128],
                    eyeb[:])
                # pt[tfine, (q2, d)] covers g = xc*16 + q2*4 + gq
                g0 = xc * (XCH // 128)
                dst = a_t[:][:, g0 * DSH:(g0 + 16) * DSH] \
                    .rearrange("p (q2 gq d) -> p q2 gq d", q2=4, gq=4)[
                        :, :, gq, :]
                psrc = pt[:].rearrange("p (q2 d) -> p q2 d", q2=4)
                nc.scalar.copy(dst, psrc)

        def precombine(b, wf_t, ztp, xwp, tmpp):
            """butterfly planes for frame batch b: zt cols (plane, qc, f, d);
            planes 0=y0, 1=y2, 2=m0, 3=m1.  Window applied via per-partition
            tensor_scalar on GpSimd; adds on DVE."""
            zt = ztp.tile([128, 4 * 4 * 512], bf16, tag="zt", name=f"zt{b}")
            for qc in range(4):
                xw = xwp.tile([128, 2048], bf16, tag="xw", name=f"xw{b}_{qc}")
                for j in range(4):
                    base = (128 * b + qc + 4 * j) * DSH
                    view = a_t[:][:, base: base + 4096] \
                        .rearrange("p (f q) -> p f q", f=16)[:, :, :DSH]
                    nc.gpsimd.tensor_scalar_mul(
                        xw[:, j * 512:(j + 1) * 512]
                        .rearrange("p (f d) -> p f d", f=16),
                        view, wcol[:, j * 4 + qc: j * 4 + qc + 1])
                tmp = tmpp.tile([128, 1024], bf16, tag="tmp", name=f"tm{b}_{qc}")
                nc.vector.tensor_add(tmp[:, :512], xw[:, 0:512], xw[:, 1024:1536])
                nc.vector.tensor_add(tmp[:, 512:], xw[:, 512:1024], xw[:, 1536:2048])
                z = lambda pl: zt[:, (pl * 4 + qc) * 512:(pl * 4 + qc + 1) * 512]
                nc.vector.tensor_sub(z(2), xw[:, 0:512], xw[:, 1024:1536])
                nc.vector.tensor_sub(z(3), xw[:, 512:1024], xw[:, 1536:2048])
                nc.vector.tensor_add(z(0), tmp[:, :512], tmp[:, 512:])
                nc.vector.tensor_sub(z(1), tmp[:, :512], tmp[:, 512:])
            return zt

        def fwd_batch(f16, wf_t, zt):
            # radix-4 forward: per m-block, accumulate plane x qc matmuls
            fc = f16
            blk = [0]
            for m in range(16):
                ps = sp.tile([128, 512], f32, tag="sm", name=f"sm{f16}_{m}")
                s = m // 4
                planes = _R4_PLANES[s]
                out_ap = ps[:].rearrange("p (d f) -> p f d", f=16)
                nmm = len(planes) * 4
                i = 0
                for pl in planes:
                    for qc in range(4):
                        rhs = zt[:, (pl * 4 + qc) * 512:(pl * 4 + qc + 1) * 512] \
                            .rearrange("p (f d) -> p f d", f=16)
                        nc.tensor.matmul(
                            out_ap,
                            wf_t[:, blk[0] * 128:(blk[0] + 1) * 128],
                            rhs,
                            start=(i == 0), stop=(i == nmm - 1))
                        blk[0] += 1
                        i += 1
                # per-m eviction (ScalarE; DVE is scanning)
                src_ = ps[:].rearrange("p (d f) -> p d f", f=16)
                doff = fc * SPECW + m * DSH * CB
                dst = spec[:][:, doff: doff + DSH * CB] \
                    .rearrange("p (d c) -> p d c", c=CB)[:, :, 1: 1 + FW]
                nc.scalar.copy(dst, src_)

        def scan_block(fc):
            # recurrence scan for frames [16fc, 16fc+16); inject copy to the
            # next block must happen BEFORE the in-place T*u multiply.
            nc.vector.tensor_tensor_scan(
                spec[:, fc * SPECW:(fc + 1) * SPECW],
                pat[:],
                spec[:, fc * SPECW:(fc + 1) * SPECW],
                0.0, Alu.mult, Alu.add)
            if fc + 1 < FC:
                src = spec[:][:, fc * SPECW: (fc + 1) * SPECW] \
                    .rearrange("p (md c) -> p md c", c=CB)[:, :, CB - 1: CB]
                dst = spec[:][:, (fc + 1) * SPECW: (fc + 2) * SPECW] \
                    .rearrange("p (md c) -> p md c", c=CB)[:, :, 0:1]
                nc.vector.tensor_copy(dst, src)
            nc.vector.tensor_mul(
                spec[:, fc * SPECW:(fc + 1) * SPECW],
                spec[:, fc * SPECW:(fc + 1) * SPECW],
                pat[:])

        # ================= phase F (+ scan), pipelined =================
        with ExitStack() as ctxF:
            wp = ctxF.enter_context(tc.tile_pool(name="wfp", bufs=1))
            sp = ctxF.enter_context(tc.tile_pool(name="sp", bufs=2, space="PSUM"))
            wf_t = wp.tile([128, 96 * 128], bf16, tag="wf")
            a_t = wp.tile([128, GPAD * DSH], bf16, tag="a")
            nc.vector.memset(a_t[:, GCH * DSH:], 0.0)
            ztp = ctxF.enter_context(tc.tile_pool(name="ztp", bufs=2))
            xwp = ctxF.enter_context(tc.tile_pool(name="xwp", bufs=2))
            tmpp = ctxF.enter_context(tc.tile_pool(name="tmpp", bufs=2))

            for xc in range(9):
                mixer_chunk(xc)
            # weights AFTER the first x chunks on the queue: mixer starts
            # immediately, wf4/pat still arrive before first use
            nc.sync.dma_start(out=wf_t[:], in_=wf4[:])
            nc.sync.dma_start(out=pat[:], in_=patd[:])
            zt0 = precombine(0, wf_t, ztp, xwp, tmpp)
            for xc in range(9, 17):
                mixer_chunk(xc)
            zt1 = precombine(1, wf_t, ztp, xwp, tmpp)
            fwd_batch(0, wf_t, zt0)
            for xc in range(17, 25):
                mixer_chunk(xc)
            scan_block(0)
            zt2 = precombine(2, wf_t, ztp, xwp, tmpp)
            fwd_batch(1, wf_t, zt1)
            for xc in range(25, NXC):
                mixer_chunk(xc)
            scan_block(1)
            zt3 = precombine(3, wf_t, ztp, xwp, tmpp)
            fwd_batch(2, wf_t, zt2)
            scan_block(2)
            fwd_batch(3, wf_t, zt3)
            scan_block(3)

        # ================= phase I (radix-4 inverse) =================
        with ExitStack() as ctxI:
            wp2 = ctxI.enter_context(tc.tile_pool(name="wip", bufs=1))
            wi_t = wp2.tile([128, 96 * 128], bf16, tag="wi")
            nc.sync.dma_start(out=wi_t[:, :48 * 128], in_=wi4d[:, :48 * 128])
            nc.scalar.dma_start(out=wi_t[:, 48 * 128:], in_=wi4d[:, 48 * 128:])
            wicol = wp2.tile([128, 16], f32, tag="wicol")
            nc.sync.dma_start(out=wicol[:], in_=wicold[:])
            ztail = wp2.tile([128, 2 * 4 * DSH], bf16, tag="ztail")
            nc.vector.memset(ztail[:], 0.0)

            tout = ctxI.enter_context(tc.tile_pool(name="tout", bufs=6))
            stg = ctxI.enter_context(tc.tile_pool(name="stg", bufs=3))
            zbp = ctxI.enter_context(tc.tile_pool(name="zbp", bufs=4))
            efp = ctxI.enter_context(tc.tile_pool(name="efp", bufs=3))
            aqp = ctxI.enter_context(tc.tile_pool(name="aqp", bufs=3))
            ywp = ctxI.enter_context(tc.tile_pool(name="ywp", bufs=3))
            ohp = ctxI.enter_context(tc.tile_pool(name="ohp", bufs=3))
            tailp = ctxI.enter_context(tc.tile_pool(name="tailp", bufs=2))

            yv = yout.rearrange("d (a4 fl t) -> fl d a4 t", fl=4, t=1024)

            def emit_store(tt, fc, qc, h):
                p4 = ppB.tile([128, 512], f32, tag="pp",
                              name=f"t4_{fc}_{qc}_{h}")
                for r2 in range(4):
                    nc.tensor.transpose(
                        p4[:, r2 * 128:(r2 + 1) * 128],
                        tt[:, r2 * 128:(r2 + 1) * 128],
                        eye[:])
                st = stg.tile([128, 512], f32, tag="stg",
                              name=f"stg{fc}_{qc}_{h}")
                if h == 0:
                    nc.vector.tensor_copy(st[:], p4[:])
                else:
                    nc.scalar.copy(st[:], p4[:])
                deng = nc.sync if h == 0 else nc.scalar
                for r2 in range(4):
                    dst = yv[:, :, 4 * fc + r2,
                             512 * h + 128 * qc: 512 * h + 128 * qc + 128]
                    deng.dma_start(
                        out=dst,
                        in_=st[:, r2 * 128:(r2 + 1) * 128])

            deferred = []
            tail_prev = ztail
            for fc in range(FC):
                tail_new = tailp.tile([128, 2 * 4 * DSH], bf16, tag="tail",
                                      name=f"tail{fc}") if fc < FC - 1 else None
                for qc in range(4):
                    # 6 z-plane transforms: contraction over family slot-reals
                    zb = zbp.tile([128, 6 * 512], bf16, tag="zb",
                                  name=f"zb{fc}_{qc}")
                    for pl in range(6):
                        s = _INV_PLANES[pl][0]
                        ps = ppA.tile([128, 512], f32, tag="pp",
                                      name=f"zp{fc}_{qc}_{pl}")
                        out_ap = ps[:].rearrange("p (f d) -> p d f", f=FW)
                        for subm in range(4):
                            m = 4 * s + subm
                            base = fc * SPECW + m * DSH * CB
                            rhs = spec[:][:, base: base + DSH * CB] \
                                .rearrange("p (d c) -> p d c", c=CB)[:, :, 1: 1 + FW]
                            blk = (qc * 6 + pl) * 4 + subm
                            nc.tensor.matmul(
                                out_ap,
                                wi_t[:, blk * 128:(blk + 1) * 128],
                                rhs, start=(subm == 0), stop=(subm == 3))
                        nc.scalar.copy(zb[:, pl * 512:(pl + 1) * 512], ps[:])
                    # butterflies (DVE, bf16): e,f,gg,h then quarters a0..a3
                    ef = efp.tile([128, 4 * 512], bf16, tag="ef",
                                  name=f"ef{fc}_{qc}")
                    z = lambda pl: zb[:, pl * 512:(pl + 1) * 512]
                    nc.vector.tensor_add(ef[:, 0 * 512:1 * 512], z(0), z(3))   # e
                    nc.vector.tensor_sub(ef[:, 1 * 512:2 * 512], z(0), z(3))   # f
                    nc.vector.tensor_add(ef[:, 2 * 512:3 * 512], z(1), z(4))   # gg
                    nc.vector.tensor_sub(ef[:, 3 * 512:4 * 512], z(5), z(2))   # h
                    aq = aqp.tile([128, 4 * 512], bf16, tag="aq",
                                  name=f"aq{fc}_{qc}")
                    E, F_, G, H = (ef[:, i * 512:(i + 1) * 512] for i in range(4))
                    nc.vector.tensor_add(aq[:, 0 * 512:1 * 512], E, G)   # a0
                    nc.vector.tensor_add(aq[:, 1 * 512:2 * 512], F_, H)  # a1
                    nc.vector.tensor_sub(aq[:, 2 * 512:3 * 512], E, G)   # a2
                    nc.vector.tensor_sub(aq[:, 3 * 512:4 * 512], F_, H)  # a3
                    # save pre-window tail quarters (a2,a3 of frame 15)
                    if tail_new is not None:
                        for j2 in range(2):
                            nc.vector.tensor_copy(
                                tail_new[:, (j2 * 4 + qc) * DSH:
                                         (j2 * 4 + qc + 1) * DSH],
                                aq[:, (2 + j2) * 512 + 15 * DSH:
                                   (2 + j2) * 512 + 16 * DSH])
                    # window (GpSimd, per-partition scalars) + OLA + tanh
                    for h in range(2):
                        yw = ywp.tile([128, 1024], f32, tag="yw",
                                      name=f"yw{fc}_{qc}_{h}")
                        nc.gpsimd.tensor_scalar_mul(
                            yw[:, :512], aq[:, h * 512:(h + 1) * 512],
                            wicol[:, h * 4 + qc: h * 4 + qc + 1])
                        nc.gpsimd.tensor_scalar_mul(
                            yw[:, 512:], aq[:, (h + 2) * 512:(h + 3) * 512],
                            wicol[:, (h + 2) * 4 + qc: (h + 2) * 4 + qc + 1])
                        # windowed tail quarter for frame 0 of this batch
                        wt = ywp.tile([128, DSH], f32, tag="wt",
                                      name=f"wt{fc}_{qc}_{h}")
                        nc.gpsimd.tensor_scalar_mul(
                            wt[:], tail_prev[:, (h * 4 + qc) * DSH:
                                             (h * 4 + qc + 1) * DSH],
                            wicol[:, (h + 2) * 4 + qc: (h + 2) * 4 + qc + 1])
                        oh = ohp.tile([128, 512], f32, tag="oh",
                                      name=f"oh{fc}_{qc}_{h}")
                        nc.vector.tensor_add(
                            oh[:, DSH:], yw[:, DSH:512], yw[:, 512:1024 - DSH])
                        nc.vector.tensor_add(oh[:, :DSH], yw[:, :DSH], wt[:])
                        # tanh now; corner-turn/store deferred 2 qc-groups
                        tt = tout.tile([128, 512], f32, tag="to",
                                       name=f"to{fc}_{qc}_{h}")
                        nc.scalar.activation(
                            tt[:], oh[:], mybir.ActivationFunctionType.Tanh)
                        deferred.append((tt, fc, qc, h))
                    while len(deferred) > 4:
                        emit_store(*deferred.pop(0))
                tail_prev = tail_new if tail_new is not None else ztail
            while deferred:
                emit_store(*deferred.pop(0))
    nc.compile()
    return nc


def _get_nc():
    global _CACHED_NC
    if _CACHED_NC is None:
        _CACHED_NC = _build_program()
    return _CACHED_NC


def kernel(x, transfer, mixer_matrix, gain, _trace=False):
    import ml_dtypes
    from concourse.bass_utils import run_bass_kernel_spmd

    x = np.ascontiguousarray(np.asarray(x, np.float32))
    transfer = np.asarray(transfer, np.float32)
    mixer_matrix = np.asarray(mixer_matrix, np.float32)
    gain = np.asarray(gain, np.float32)

    bf = ml_dtypes.bfloat16
    wf4_np = build_wf4().astype(bf)
    wcol_np = build_wcol().astype(np.float32)
    wi4_np = build_wi4().astype(bf)
    wicol_np = (float(gain.ravel()[0]) * build_wcol()).astype(np.float32)
    Ts = build_t_slots(transfer)
    eye = np.eye(128, dtype=np.float32)
    eyeb = np.eye(128, dtype=np.float64).astype(bf)

    in_maps = []
    for c in range(8):
        b, dh = c // 2, c % 2
        mixw = mixer_matrix[:, dh * DSH:(dh + 1) * DSH].astype(bf)
        patc = build_pattern(Ts[dh * DSH:(dh + 1) * DSH]).astype(bf)
        in_maps.append({
            "xb": x[b].astype(bf),
            "mixw": mixw,
            "wf4": wf4_np,
            "wcol": wcol_np,
            "wi4": wi4_np,
            "wicol": wicol_np,
            "pat": patc,
            "eye": eye,
            "eyeb": eyeb,
        })

    nc = _get_nc()
    res = run_bass_kernel_spmd(nc, in_maps, list(range(8)), trace=_trace)
    out = np.zeros((BATCH, CPD, TIME), np.float32)
    for c in range(8):
        b, dh = c // 2, c % 2
        out[b, dh * DSH:(dh + 1) * DSH] = res.results[c]["y"]
    if _trace:
        return out, res
    return out


# revision 22
# speedup vs baseline: 1.1522x; 1.0112x over previous
"""Trainium2 Bass kernel for nn_Block_38517266710836.

reference pipeline: channel mixer -> STFT (hann 2048, hop 1024) -> per-frame
recurrence out[f] = (spec[f] + out[f-1]) * transfer -> iSTFT (hann synthesis)
-> overlap-add -> gain -> tanh.

Sharding: 8 cores, data-parallel over (batch, channel-half): core c handles
batch c//2, mixed channels [32*(c%2), +32). Each core receives its batch's
full 64-channel input (the mixer contracts channels) and writes 32 rows.

Pipelined single-pass program per core: mixer chunks, forward-DFT frame
batches, the DVE recurrence scan, and the inverse/overlap-add phase are
interleaved so the scan and evictions hide under PE matmul work.  Forward
evictions and corner-turn copies run on ScalarE (DVE is reserved for the
scan), weights stream on the gpsimd DMA queue, x/y on the sync queue, and
PSUM pools are shared across phases to fit the 8-bank budget.
"""

import numpy as np

WINDOW = 2048
STEP = 1024
CPD = 64
BATCH = 4
TIME = 65536
FRAMES = 64
NJ = 16              # per-frame time chunks (fwd contraction blocks)
NM = 16              # spectral slot chunks
DSH = 32             # mixed channels per core
GCH = TIME // 128    # 512 global 128-sample chunks
GPAD = GCH + 16      # + zero pad (frame 63 reaches t=66560; extra width so
                     # the forward rhs slice [base, base+2048) stays in-bounds)
FC = 4               # frame chunks for the scan layout
FW = 16              # frames per chunk
CB = 17              # chain block: 1 inject/reset col + 16 frame cols
SPECW = NM * DSH * CB  # 8704 free cols per fc block


def _hann(n):
    return (0.5 - 0.5 * np.cos(2.0 * np.pi * np.arange(n) / n)).astype(np.float64)


def _slot_tables():
    """slot s in [0,2048): s<1024 -> Re[k=s]; s==1024 -> Re[1024] (parked in
    Im[0]'s slot, since Im[0] is identically 0); s>1024 -> Im[k=s-1024]."""
    k_of_slot = np.zeros(2048, np.int64)
    is_im = np.zeros(2048, np.bool_)
    for s in range(2048):
        if s < 1024:
            k_of_slot[s] = s
        elif s == 1024:
            k_of_slot[s] = 1024
        else:
            k_of_slot[s] = s - 1024
            is_im[s] = True
    return k_of_slot, is_im


def build_fwd_weights():
    """[2048 n, 2048 slots]: windowed rfft of one frame, slot layout."""
    n = np.arange(WINDOW, dtype=np.float64)
    w = _hann(WINDOW)
    k_of_slot, is_im = _slot_tables()
    ang = 2.0 * np.pi * np.outer(n, k_of_slot.astype(np.float64)) / WINDOW
    W = np.where(is_im[None, :], -np.sin(ang), np.cos(ang))
    W *= w[:, None]
    return W


def build_inv_weights(gain):
    """[2048 slots, 2048 n]: gain * hann * irfft from slot layout."""
    n = np.arange(WINDOW, dtype=np.float64)
    w = _hann(WINDOW)
    k_of_slot, is_im = _slot_tables()
    ang = 2.0 * np.pi * np.outer(k_of_slot.astype(np.float64), n) / WINDOW
    k = k_of_slot
    re_coef = (2.0 - (k == 0) - (k == 1024))[:, None] / WINDOW * np.cos(ang)
    im_coef = -2.0 / WINDOW * np.sin(ang)
    W = np.where(is_im[:, None], im_coef, re_coef)
    W[1024, :] = np.cos(np.pi * n) / WINDOW
    W *= (gain * w)[None, :]
    return W


def _slot_tables_r4():
    """family-major slot layout: slot' = s*512 + local; family s holds
    k = s, s+4, ... <= 1024(ish), (re, im) interleaved k-major."""
    karr = np.zeros(2048, np.int64)
    isim = np.zeros(2048, np.bool_)
    pos = 0
    for s in range(4):
        for k in range(s, 1025, 4):
            karr[pos] = k; isim[pos] = False; pos += 1
            if k not in (0, 1024):
                karr[pos] = k; isim[pos] = True; pos += 1
    assert pos == 2048
    return karr, isim


_R4_PLANES = {0: [0], 1: [2, 3], 2: [1], 3: [2, 3]}  # m//4 -> plane list


def _build_wfam():
    """family -> list of (plane, [512 q, 512 r]) weight matrices.
    planes: 0=y0, 1=y2, 2=m0, 3=m1."""
    q = np.arange(512, dtype=np.float64)
    karr, isim = _slot_tables_r4()
    fams = {}
    for s in range(4):
        kv = karr[s * 512:(s + 1) * 512].astype(np.float64)
        iv = isim[s * 512:(s + 1) * 512]
        ang = 2.0 * np.pi * np.outer(q, kv) / WINDOW
        c, sn = np.cos(ang), np.sin(ang)
        if s == 0:
            fams[s] = [(0, np.where(iv[None, :], -sn, c))]
        elif s == 2:
            fams[s] = [(1, np.where(iv[None, :], -sn, c))]
        elif s == 1:
            fams[s] = [(2, np.where(iv[None, :], -sn, c)),
                       (3, np.where(iv[None, :], -c, -sn))]
        else:
            fams[s] = [(2, np.where(iv[None, :], -sn, c)),
                       (3, np.where(iv[None, :], c, sn))]
    return fams


def build_wf4():
    """[128, 96*128] SBUF-ready block layout matching the device MM loop:
    for qp, mi: m=2qp+mi -> (s=m//4, subm=m%4): for plane, for qc: block
    = Wfam[s][plane][qc*128:+128, subm*128:+128]."""
    fams = _build_wfam()
    blocks = []
    for qp in range(8):
        for mi in range(2):
            m = qp * 2 + mi
            s, subm = m // 4, m % 4
            for pl, Wm in fams[s]:
                for qc in range(4):
                    blocks.append(Wm[qc * 128:(qc + 1) * 128,
                                     subm * 128:(subm + 1) * 128])
    return np.concatenate(blocks, axis=1)  # [128, 96*128]


def build_wcol():
    """[128, 16] per-partition window scalars: col j*4+qc = w[qc*128+p+512j]."""
    w = _hann(WINDOW)
    out = np.zeros((128, 16), np.float64)
    for j in range(4):
        for qc in range(4):
            out[:, j * 4 + qc] = w[qc * 128 + np.arange(128) + 512 * j]
    return out


def build_t_slots(transfer):
    karr, _ = _slot_tables_r4()
    return np.asarray(transfer, np.float64)[:, karr]  # [ch, 2048]


_INV_PLANES = [(0, False), (1, False), (1, True), (2, False), (3, False), (3, True)]
# zb plane order: 0=zre0, 1=zre1, 2=zim1, 3=zre2, 4=zre3, 5=zim3


def build_wi4():
    """[128, 96*128] inverse z-plane weights; device order:
    for qc in 4: for pl in 6: for subm in 4."""
    karr, isim = _slot_tables_r4()
    q = np.arange(512, dtype=np.float64)
    Vs = []
    for (s, want_im) in _INV_PLANES:
        kv = karr[s * 512:(s + 1) * 512].astype(np.float64)
        iv = isim[s * 512:(s + 1) * 512]
        coef = (2.0 - (kv == 0) - (kv == 1024)) / WINDOW
        ang = 2.0 * np.pi * np.outer(kv, q) / WINDOW
        c, sn = np.cos(ang), np.sin(ang)
        V = coef[:, None] * (np.where(iv[:, None], c, sn) if want_im
                             else np.where(iv[:, None], -sn, c))
        Vs.append(V)  # [512 slot-reals, 512 q]
    blocks = []
    for qc in range(4):
        for V in Vs:
            for subm in range(4):
                blocks.append(V[subm * 128:(subm + 1) * 128,
                               qc * 128:(qc + 1) * 128])
    return np.concatenate(blocks, axis=1)


def build_inv_weights_perm(gain):
    """[2048 slots', 2048 n]: gain * hann * irfft from the r4 slot layout."""
    n = np.arange(WINDOW, dtype=np.float64)
    w = _hann(WINDOW)
    karr, isim = _slot_tables_r4()
    k = karr.astype(np.float64)
    ang = 2.0 * np.pi * np.outer(k, n) / WINDOW
    re_coef = (2.0 - (karr == 0) - (karr == 1024))[:, None] / WINDOW * np.cos(ang)
    im_coef = -2.0 / WINDOW * np.sin(ang)
    W = np.where(isim[:, None], im_coef, re_coef)
    W *= (gain * w)[None, :]
    return W


def build_fwd_weights_perm():
    """effective [2048 n, 2048 slots'] fwd matrix (validation only)."""
    n = np.arange(WINDOW, dtype=np.float64)
    w = _hann(WINDOW)
    karr, isim = _slot_tables_r4()
    ang = 2.0 * np.pi * np.outer(n, karr.astype(np.float64)) / WINDOW
    W = np.where(isim[None, :], -np.sin(ang), np.cos(ang))
    W *= w[:, None]
    return W


def build_pattern(t_slots_core):
    """T-pattern [128, SPECW]: per (m,d) chain block of CB cols:
    col 0 = 0 (reset/inject), cols 1..16 = T[slot(m,kf), d]."""
    pat = np.zeros((128, SPECW), np.float64)
    for m in range(NM):
        for d in range(DSH):
            base = (m * DSH + d) * CB
            pat[:, base + 1: base + CB] = \
                t_slots_core[d, m * 128:(m + 1) * 128][:, None]
    return pat


def emulate(x, transfer, mixer_matrix, gain, wdtype=np.float32):
    """Numpy emulation of the device math (offline validation)."""
    b, c, t = x.shape
    Wf = build_fwd_weights_perm().astype(wdtype).astype(np.float64)
    Wi = build_inv_weights_perm(float(np.asarray(gain).ravel()[0])).astype(wdtype).astype(np.float64)
    Ts = build_t_slots(transfer)
    y = np.einsum('bct,cd->bdt', np.asarray(x, np.float64),
                  np.asarray(mixer_matrix, np.float64))
    yp = np.pad(y, ((0, 0), (0, 0), (0, STEP)))
    out = np.zeros((b, c, t), np.float64)
    for bi in range(b):
        frames = np.stack([yp[bi, :, f * STEP: f * STEP + WINDOW]
                           for f in range(FRAMES)], 1)
        spec = frames.astype(wdtype).astype(np.float64) @ Wf
        st = np.zeros((c, 2048))
        outs = np.zeros_like(spec)
        for f in range(FRAMES):
            st = (spec[:, f].astype(wdtype).astype(np.float64) + st) * Ts
            outs[:, f] = st
        aud = outs.astype(wdtype).astype(np.float64) @ Wi
        acc = np.zeros((c, t + STEP))
        for f in range(FRAMES):
            acc[:, f * STEP: f * STEP + WINDOW] += aud[:, f]
        out[bi] = np.tanh(acc[:, :t])
    return out.astype(np.float32)


# ---------------------------------------------------------------------------
# Device program
# ---------------------------------------------------------------------------

_CACHED_NC = None


def _build_program():
    import concourse.bacc as bacc
    import concourse.mybir as mybir
    from concourse import tile
    from contextlib import ExitStack

    f32 = mybir.dt.float32
    bf16 = mybir.dt.bfloat16
    Alu = mybir.AluOpType

    nc = bacc.Bacc("TRN2", target_bir_lowering=False, debug=False, num_devices=8)
    xb = nc.dram_tensor("xb", [CPD, TIME], bf16, kind="ExternalInput").ap()
    mixw = nc.dram_tensor("mixw", [CPD, DSH], bf16, kind="ExternalInput").ap()
    wf4 = nc.dram_tensor("wf4", [128, 96 * 128], bf16, kind="ExternalInput").ap()
    wcold = nc.dram_tensor("wcol", [128, 16], f32, kind="ExternalInput").ap()
    wi4d = nc.dram_tensor("wi4", [128, 96 * 128], bf16, kind="ExternalInput").ap()
    wicold = nc.dram_tensor("wicol", [128, 16], f32, kind="ExternalInput").ap()
    patd = nc.dram_tensor("pat", [128, SPECW], bf16, kind="ExternalInput").ap()
    eyed = nc.dram_tensor("eye", [128, 128], f32, kind="ExternalInput").ap()
    eyebd = nc.dram_tensor("eyeb", [128, 128], bf16, kind="ExternalInput").ap()
    yout = nc.dram_tensor("y", [DSH, TIME], f32, kind="ExternalOutput").ap()

    XCH = 2048           # x streamed in [64, 2048] chunks (16 g-chunks each)
    NXC = TIME // XCH    # 32

    with tile.TileContext(nc) as tc, ExitStack() as ctx:
        persist = ctx.enter_context(tc.tile_pool(name="persist", bufs=1))
        spec = persist.tile([128, FC * SPECW], bf16, tag="spec")
        pat = persist.tile([128, SPECW], bf16, tag="pat")
        mx = persist.tile([CPD, DSH], bf16, tag="mx")
        eyeb = persist.tile([128, 128], bf16, tag="eyeb")
        eye = persist.tile([128, 128], f32, tag="eye")
        wcol = persist.tile([128, 16], f32, tag="wcol")

        # PSUM (8 banks of 2KB/partition):
        #   ppA [128,512] f32 x2 = 2 banks  (mixer psum, then phase-I OLA)
        #   ppB [128,512] f32 x2 = 2 banks  (corner-turn psum, then phase-I t4)
        #   sp  [128,1024] f32 x2 = 4 banks (fwd DFT accumulators)
        ppA = ctx.enter_context(tc.tile_pool(name="ppA", bufs=3, space="PSUM"))
        ppB = ctx.enter_context(tc.tile_pool(name="ppB", bufs=3, space="PSUM"))

        xin = ctx.enter_context(tc.tile_pool(name="xin", bufs=2))
        ymp = ctx.enter_context(tc.tile_pool(name="ymp", bufs=2))

        # small/early tensors on the sync queue; big weights on gpsimd queue
        nc.sync.dma_start(out=mx[:], in_=mixw[:])
        nc.sync.dma_start(out=eyeb[:], in_=eyebd[:])
        nc.sync.dma_start(out=wcol[:], in_=wcold[:])
        nc.sync.dma_start(out=eye[:], in_=eyed[:])

        # chain col 0 of the first fc block must read as 0 (fresh state)
        nc.vector.memset(
            spec[:][:, 0:SPECW].rearrange(
                "p (md c) -> p md c", c=CB)[:, :, 0:1], 0.0)

        def mixer_chunk(xc):
            xt = xin.tile([CPD, XCH], bf16, tag="x", name=f"x{xc}")
            nc.sync.dma_start(out=xt[:], in_=xb[:, xc * XCH:(xc + 1) * XCH])
            pm = ppA.tile([128, 512], f32, tag="pp", name=f"mix{xc}")
            for q in range(4):
                nc.tensor.matmul(
                    pm[q * DSH:(q + 1) * DSH, :],
                    mx[:],
                    xt[:, q * 512:(q + 1) * 512],
                    start=True, stop=True,
                    tile_position=(0, q * DSH))
            ym = ymp.tile([128, 512], bf16, tag="ym", name=f"ym{xc}")
            nc.scalar.copy(ym[:], pm[:])
            # ym[(q,d), tloc]: t = xc*2048 + q*512 + tloc
            for gq in range(4):  # per 4 g-chunks (one psum turn tile)
                pt = ppB.tile([128, 128], bf16, tag="pp", name=f"turn{xc}_{gq}")
                nc.tensor.transpose(
                    pt[:],
                    ym[:, gq * 128: gq * 128 + 128],
                    eyeb[:])
                # pt[tfine, (q2, d)] covers g = xc*16 + q2*4 + gq
                g0 = xc * (XCH // 128)
                dst = a_t[:][:, g0 * DSH:(g0 + 16) * DSH] \
                    .rearrange("p (q2 gq d) -> p q2 gq d", q2=4, gq=4)[
                        :, :, gq, :]
                psrc = pt[:].rearrange("p (q2 d) -> p q2 d", q2=4)
                nc.scalar.copy(dst, psrc)

        def precombine(b, wf_t, ztp, xwp, tmpp):
            """butterfly planes for frame batch b: zt cols (plane, qc, f, d);
            planes 0=y0, 1=y2, 2=m0, 3=m1.  Window applied via per-partition
            tensor_scalar on GpSimd; adds on DVE."""
            zt = ztp.tile([128, 4 * 4 * 512], bf16, tag="zt", name=f"zt{b}")
            for qc in range(4):
                xw = xwp.tile([128, 2048], bf16, tag="xw", name=f"xw{b}_{qc}")
                for j in range(4):
                    base = (128 * b + qc + 4 * j) * DSH
                    view = a_t[:][:, base: base + 4096] \
                        .rearrange("p (f q) -> p f q", f=16)[:, :, :DSH]
                    nc.gpsimd.tensor_scalar_mul(
                        xw[:, j * 512:(j + 1) * 512]
                        .rearrange("p (f d) -> p f d", f=16),
                        view, wcol[:, j * 4 + qc: j * 4 + qc + 1])
                tmp = tmpp.tile([128, 1024], bf16, tag="tmp", name=f"tm{b}_{qc}")
                nc.vector.tensor_add(tmp[:, :512], xw[:, 0:512], xw[:, 1024:1536])
                nc.vector.tensor_add(tmp[:, 512:], xw[:, 512:1024], xw[:, 1536:2048])
                z = lambda pl: zt[:, (pl * 4 + qc) * 512:(pl * 4 + qc + 1) * 512]
                nc.vector.tensor_sub(z(2), xw[:, 0:512], xw[:, 1024:1536])
                nc.vector.tensor_sub(z(3), xw[:, 512:1024], xw[:, 1536:2048])
                nc.vector.tensor_add(z(0), tmp[:, :512], tmp[:, 512:])
                nc.vector.tensor_sub(z(1), tmp[:, :512], tmp[:, 512:])
            return zt

        def fwd_batch(f16, wf_t, zt):
            # radix-4 forward: per m-block, accumulate plane x qc matmuls
            fc = f16
            blk = [0]
            for m in range(16):
                ps = sp.tile([128, 512], f32, tag="sm", name=f"sm{f16}_{m}")
                s = m // 4
                planes = _R4_PLANES[s]
                out_ap = ps[:].rearrange("p (d f) -> p f d", f=16)
                nmm = len(planes) * 4
                i = 0
                for pl in planes:
                    for qc in range(4):
                        rhs = zt[:, (pl * 4 + qc) * 512:(pl * 4 + qc + 1) * 512] \
                            .rearrange("p (f d) -> p f d", f=16)
                        nc.tensor.matmul(
                            out_ap,
                            wf_t[:, blk[0] * 128:(blk[0] + 1) * 128],
                            rhs,
                            start=(i == 0), stop=(i == nmm - 1))
                        blk[0] += 1
                        i += 1
                # per-m eviction (ScalarE; DVE is scanning)
                src_ = ps[:].rearrange("p (d f) -> p d f", f=16)
                doff = fc * SPECW + m * DSH * CB
                dst = spec[:][:, doff: doff + DSH * CB] \
                    .rearrange("p (d c) -> p d c", c=CB)[:, :, 1: 1 + FW]
                nc.scalar.copy(dst, src_)

        def scan_block(fc):
            # recurrence scan for frames [16fc, 16fc+16); inject copy to the
            # next block must happen BEFORE the in-place T*u multiply.
            nc.vector.tensor_tensor_scan(
                spec[:, fc * SPECW:(fc + 1) * SPECW],
                pat[:],
                spec[:, fc * SPECW:(fc + 1) * SPECW],
                0.0, Alu.mult, Alu.add)
            if fc + 1 < FC:
                src = spec[:][:, fc * SPECW: (fc + 1) * SPECW] \
                    .rearrange("p (md c) -> p md c", c=CB)[:, :, CB - 1: CB]
                dst = spec[:][:, (fc + 1) * SPECW: (fc + 2) * SPECW] \
                    .rearrange("p (md c) -> p md c", c=CB)[:, :, 0:1]
                nc.vector.tensor_copy(dst, src)
            nc.vector.tensor_mul(
                spec[:, fc * SPECW:(fc + 1) * SPECW],
                spec[:, fc * SPECW:(fc + 1) * SPECW],
                pat[:])

        # ================= phase F (+ scan), pipelined =================
        with ExitStack() as ctxF:
            wp = ctxF.enter_context(tc.tile_pool(name="wfp", bufs=1))
            sp = ctxF.enter_context(tc.tile_pool(name="sp", bufs=2, space="PSUM"))
            wf_t = wp.tile([128, 96 * 128], bf16, tag="wf")
            a_t = wp.tile([128, GPAD * DSH], bf16, tag="a")
            nc.vector.memset(a_t[:, GCH * DSH:], 0.0)
            ztp = ctxF.enter_context(tc.tile_pool(name="ztp", bufs=2))
            xwp = ctxF.enter_context(tc.tile_pool(name="xwp", bufs=2))
            tmpp = ctxF.enter_context(tc.tile_pool(name="tmpp", bufs=2))

            for xc in range(9):
                mixer_chunk(xc)
            # weights AFTER the first x chunks on the queue: mixer starts
            # immediately, wf4/pat still arrive before first use
            nc.sync.dma_start(out=wf_t[:], in_=wf4[:])
            nc.sync.dma_start(out=pat[:], in_=patd[:])
            zt0 = precombine(0, wf_t, ztp, xwp, tmpp)
            for xc in range(9, 17):
                mixer_chunk(xc)
            zt1 = precombine(1, wf_t, ztp, xwp, tmpp)
            fwd_batch(0, wf_t, zt0)
            for xc in range(17, 25):
                mixer_chunk(xc)
            scan_block(0)
            zt2 = precombine(2, wf_t, ztp, xwp, tmpp)
            fwd_batch(1, wf_t, zt1)
            for xc in range(25, NXC):
                mixer_chunk(xc)
            scan_block(1)
            zt3 = precombine(3, wf_t, ztp, xwp, tmpp)
            fwd_batch(2, wf_t, zt2)
            scan_block(2)
            fwd_batch(3, wf_t, zt3)
            scan_block(3)

        # ================= phase I (radix-4 inverse) =================
        with ExitStack() as ctxI:
            wp2 = ctxI.enter_context(tc.tile_pool(name="wip", bufs=1))
            wi_t = wp2.tile([128, 96 * 128], bf16, tag="wi")
            nc.sync.dma_start(out=wi_t[:, :48 * 128], in_=wi4d[:, :48 * 128])
            nc.scalar.dma_start(out=wi_t[:, 48 * 128:], in_=wi4d[:, 48 * 128:])
            wicol = wp2.tile([128, 16], f32, tag="wicol")
            nc.sync.dma_start(out=wicol[:], in_=wicold[:])
            ztail = wp2.tile([128, 2 * 4 * DSH], bf16, tag="ztail")
            nc.vector.memset(ztail[:], 0.0)

            tout = ctxI.enter_context(tc.tile_pool(name="tout", bufs=6))
            stg = ctxI.enter_context(tc.tile_pool(name="stg", bufs=3))
            zbp = ctxI.enter_context(tc.tile_pool(name="zbp", bufs=3))
            efp = ctxI.enter_context(tc.tile_pool(name="efp", bufs=3))
            aqp = ctxI.enter_context(tc.tile_pool(name="aqp", bufs=3))
            ywp = ctxI.enter_context(tc.tile_pool(name="ywp", bufs=3))
            ohp = ctxI.enter_context(tc.tile_pool(name="ohp", bufs=3))
            tailp = ctxI.enter_context(tc.tile_pool(name="tailp", bufs=2))

            yv = yout.rearrange("d (a4 fl t) -> fl d a4 t", fl=4, t=1024)

            def emit_store(tt, fc, qc, h):
                p4 = ppB.tile([128, 512], f32, tag="pp",
                              name=f"t4_{fc}_{qc}_{h}")
                for r2 in range(4):
                    nc.tensor.transpose(
                        p4[:, r2 * 128:(r2 + 1) * 128],
                        tt[:, r2 * 128:(r2 + 1) * 128],
                        eye[:])
                st = stg.tile([128, 512], f32, tag="stg",
                              name=f"stg{fc}_{qc}_{h}")
                if h == 0:
                    nc.vector.tensor_copy(st[:], p4[:])
                else:
                    nc.scalar.copy(st[:], p4[:])
                deng = nc.sync if h == 0 else nc.scalar
                for r2 in range(4):
                    dst = yv[:, :, 4 * fc + r2,
                             512 * h + 128 * qc: 512 * h + 128 * qc + 128]
                    deng.dma_start(
                        out=dst,
                        in_=st[:, r2 * 128:(r2 + 1) * 128])

            deferred = []
            tail_prev = ztail
            for fc in range(FC):
                tail_new = tailp.tile([128, 2 * 4 * DSH], bf16, tag="tail",
                                      name=f"tail{fc}") if fc < FC - 1 else None
                for qc in range(4):
                    # 6 z-plane transforms: contraction over family slot-reals
                    zb = zbp.tile([128, 6 * 512], bf16, tag="zb",
                                  name=f"zb{fc}_{qc}")
                    for pl in range(6):
                        s = _INV_PLANES[pl][0]
                        ps = ppA.tile([128, 512], f32, tag="pp",
                                      name=f"zp{fc}_{qc}_{pl}")
                        out_ap = ps[:].rearrange("p (f d) -> p d f", f=FW)
                        for subm in range(4):
                            m = 4 * s + subm
                            base = fc * SPECW + m * DSH * CB
                            rhs = spec[:][:, base: base + DSH * CB] \
                                .rearrange("p (d c) -> p d c", c=CB)[:, :, 1: 1 + FW]
                            blk = (qc * 6 + pl) * 4 + subm
                            nc.tensor.matmul(
                                out_ap,
                                wi_t[:, blk * 128:(blk + 1) * 128],
                                rhs, start=(subm == 0), stop=(subm == 3))
                        if pl % 2 == 0:
                            nc.scalar.copy(zb[:, pl * 512:(pl + 1) * 512], ps[:])
                        else:
                            nc.vector.tensor_copy(
                                zb[:, pl * 512:(pl + 1) * 512], ps[:])
                    # butterflies (DVE, bf16): e,f,gg,h then quarters a0..a3
                    ef = efp.tile([128, 4 * 512], bf16, tag="ef",
                                  name=f"ef{fc}_{qc}")
                    z = lambda pl: zb[:, pl * 512:(pl + 1) * 512]
                    nc.vector.tensor_add(ef[:, 0 * 512:1 * 512], z(0), z(3))   # e
                    nc.vector.tensor_sub(ef[:, 1 * 512:2 * 512], z(0), z(3))   # f
                    nc.vector.tensor_add(ef[:, 2 * 512:3 * 512], z(1), z(4))   # gg
                    nc.vector.tensor_sub(ef[:, 3 * 512:4 * 512], z(5), z(2))   # h
                    aq = aqp.tile([128, 4 * 512], bf16, tag="aq",
                                  name=f"aq{fc}_{qc}")
                    E, F_, G, H = (ef[:, i * 512:(i + 1) * 512] for i in range(4))
                    nc.vector.tensor_add(aq[:, 0 * 512:1 * 512], E, G)   # a0
                    nc.vector.tensor_add(aq[:, 1 * 512:2 * 512], F_, H)  # a1
                    nc.vector.tensor_sub(aq[:, 2 * 512:3 * 512], E, G)   # a2
                    nc.vector.tensor_sub(aq[:, 3 * 512:4 * 512], F_, H)  # a3
                    # save pre-window tail quarters (a2,a3 of frame 15)
                    if tail_new is not None:
                        for j2 in range(2):
                            nc.vector.tensor_copy(
                                tail_new[:, (j2 * 4 + qc) * DSH:
                                         (j2 * 4 + qc + 1) * DSH],
                                aq[:, (2 + j2) * 512 + 15 * DSH:
                                   (2 + j2) * 512 + 16 * DSH])
                    # window (GpSimd, per-partition scalars) + OLA + tanh
                    for h in range(2):
                        yw = ywp.tile([128, 1024], f32, tag="yw",
                                      name=f"yw{fc}_{qc}_{h}")
                        nc.gpsimd.tensor_scalar_mul(
                            yw[:, :512], aq[:, h * 512:(h + 1) * 512],
                            wicol[:, h * 4 + qc: h * 4 + qc + 1])
                        nc.gpsimd.tensor_scalar_mul(
                            yw[:, 512:], aq[:, (h + 2) * 512:(h + 3) * 512],
                            wicol[:, (h + 2) * 4 + qc: (h + 2) * 4 + qc + 1])
                        # windowed tail quarter for frame 0 of this batch
                        wt = ywp.tile([128, DSH], f32, tag="wt",
                                      name=f"wt{fc}_{qc}_{h}")
                        nc.gpsimd.tensor_scalar_mul(
                            wt[:], tail_prev[:, (h * 4 + qc) * DSH:
                                             (h * 4 + qc + 1) * DSH],
                            wicol[:, (h + 2) * 4 + qc: (h + 2) * 4 + qc + 1])
                        oh = ohp.tile([128, 512], f32, tag="oh",
                                      name=f"oh{fc}_{qc}_{h}")
                        nc.vector.tensor_add(
                            oh[:, DSH:], yw[:, DSH:512], yw[:, 512:1024 - DSH])
                        nc.vector.tensor_add(oh[:, :DSH], yw[:, :DSH], wt[:])
                        # tanh now; corner-turn/store deferred 2 qc-groups
                        tt = tout.tile([128, 512], f32, tag="to",
                                       name=f"to{fc}_{qc}_{h}")
                        nc.scalar.activation(
                            tt[:], oh[:], mybir.ActivationFunctionType.Tanh)
                        deferred.append((tt, fc, qc, h))
                    while len(deferred) > 4:
                        emit_store(*deferred.pop(0))
                tail_prev = tail_new if tail_new is not None else ztail
            while deferred:
                emit_store(*deferred.pop(0))
    nc.compile()
    return nc


def _get_nc():
    global _CACHED_NC
    if _CACHED_NC is None:
        _CACHED_NC = _build_program()
    return _CACHED_NC


def kernel(x, transfer, mixer_matrix, gain, _trace=False):
    import ml_dtypes
    from concourse.bass_utils import run_bass_kernel_spmd

    x = np.ascontiguousarray(np.asarray(x, np.float32))
    transfer = np.asarray(transfer, np.float32)
    mixer_matrix = np.asarray(mixer_matrix, np.float32)
    gain = np.asarray(gain, np.float32)

    bf = ml_dtypes.bfloat16
    wf4_np = build_wf4().astype(bf)
    wcol_np = build_wcol().astype(np.float32)
    wi4_np = build_wi4().astype(bf)
    wicol_np = (float(gain.ravel()[0]) * build_wcol()).astype(np.float32)
    Ts = build_t_slots(transfer)
    eye = np.eye(128, dtype=np.float32)
    eyeb = np.eye(128, dtype=np.float64).astype(bf)

    in_maps = []
    for c in range(8):
        b, dh = c // 2, c % 2
        mixw = mixer_matrix[:, dh * DSH:(dh + 1) * DSH].astype(bf)
        patc = build_pattern(Ts[dh * DSH:(dh + 1) * DSH]).astype(bf)
        in_maps.append({
            "xb": x[b].astype(bf),
            "mixw": mixw,
            "wf4": wf4_np,
            "wcol": wcol_np,
            "wi4": wi4_np,
            "wicol": wicol_np,
            "pat": patc,
            "eye": eye,
            "eyeb": eyeb,
        })

    nc = _get_nc()
    res = run_bass_kernel_spmd(nc, in_maps, list(range(8)), trace=_trace)
    out = np.zeros((BATCH, CPD, TIME), np.float32)
    for c in range(8):
        b, dh = c // 2, c % 2
        out[b, dh * DSH:(dh + 1) * DSH] = res.results[c]["y"]
    if _trace:
        return out, res
    return out
